# revision 19
# baseline (speedup 1.0000x reference)
"""Catmull-Rom spline loss kernel for Trainium2 (8 NeuronCores, data-parallel).

Math: out[n,c] = sum_ij wx_i wy_j CP[a+i-1, b+j-1, c] with wx = [x^3,x^2,x,1]@A.
Rewritten in the monomial basis: out[n,c] = sum_{p,q} x^p y^q G_pq[a,b,c] where
G_pq[a,b,c] = sum_ij B[p,i] B[q,j] CP[a-1+i, b-1+j, c] and B = A rows reversed.
G is precomputed on-device from CP (it is cell-indexed: a,b in [1,61]), stored
as a [61,61,32] DRAM table (minor dim = p*8 + q*2 + c). Each point then needs a
single 128-byte row gather (indirect DMA) + polynomial evaluation (DVE).
loss = sum_n |ch1_n - out_n|^2 accumulated per partition, reduced on host.
"""

import os

os.environ.setdefault("MYCRO_LOCAL_CACHE", "1")

import numpy as np

import bass_rust
import concourse.bass as bass
import concourse.mybir as mybir
import concourse.tile as tile
import concourse.bacc as bacc
import types
from concourse import library_config
from concourse.library_overlay import lower_extended_insts
from concourse.vector_clock import ScopedClock
from concourse.masks import make_identity
from concourse.bass_utils import run_bass_kernel_spmd

F32 = mybir.dt.float32
I32 = mybir.dt.int32
I16 = mybir.dt.int16
ALU = mybir.AluOpType

NCORES = 8
P = 128
G = 64
NCELL = 61  # valid index range [1, 61] -> 61 cells per axis
GSUB = 8  # gather sub-call rows (1024 indices) - descriptor ring limit

# Catmull-Rom basis (same as reference.py); B[p] = A[3-p] so that
# wx_i = sum_p B[p, i] * x^p.
A_MAT = np.array(
    [[-0.5, 1.5, -1.5, 0.5],
     [1.0, -2.5, 2.0, -0.5],
     [-0.5, 0.0, 0.5, 0.0],
     [0.0, 1.0, 0.0, 0.0]], dtype=np.float64)
B_MAT = A_MAT[::-1, :]

_MAX_WAITS = 1


def _split_multiwait(nc, max_waits=_MAX_WAITS):
    """The walrus snapshot here rejects instructions carrying more than one
    sync wait; move extra waits onto injected same-engine NoOps."""
    n_split = 0
    for bb in nc.main_func.blocks:
        insts = bb.instructions
        new = []
        for ins in insts:
            si = ins.sync_info
            waits = list(si.on_wait) if si and si.on_wait else []
            if len(waits) > max_waits:
                extra, keep = waits[:-max_waits], waits[-max_waits:]
                for k in range(0, len(extra), max_waits):
                    nop = mybir.InstDrain(
                        name=f"{ins.name}-wsplit{k}", ins=[], outs=[])
                    nop.engine = ins.engine
                    nop.sync_info = bass_rust.SyncInfo(
                        on_wait=extra[k:k + max_waits], on_update=[])
                    new.append(nop)
                ins.sync_info = bass_rust.SyncInfo(
                    on_wait=keep,
                    on_update=list(si.on_update) if si.on_update else [])
                n_split += 1
            new.append(ins)
        insts[:] = new
    return n_split


def _emit_precompute(nc, tc, sbuf, psum, gall, cpt_d):  # noqa: C901
    """Build G[61,61,32] fp32 in DRAM from the cpt input ([128,64] = CP
    transposed so partition p = 2*s + c holds row-series CP[:, s, c])."""
    ident = sbuf.tile([P, P], F32, tag="ident")
    make_identity(nc, ident[:])

    # zero-fill the 256B-row padding (cols 32:64) once; the gather reads
    # full rows and the simulator requires finite values.
    zt = sbuf.tile([NCELL, NCELL * 16], F32, tag="zpad")
    nc.gpsimd.memset(zt[:], 0.0)
    nc.sync.dma_start(
        out=gall[:, :, 32:48],
        in_=zt[:].rearrange("p (b m) -> p b m", m=16))
    nc.sync.dma_start(
        out=gall[:, :, 48:64],
        in_=zt[:].rearrange("p (b m) -> p b m", m=16))

    cpt = sbuf.tile([P, G], F32, tag="cpt")
    nc.sync.dma_start(out=cpt[:], in_=cpt_d[:])

    # Pass 1 (contract row-offset i): H[(s,c), p*61 + (a-1)]
    h = sbuf.tile([P, 4 * NCELL], F32, tag="h")
    t0 = sbuf.tile([P, NCELL], F32, tag="pre_t0")
    t1 = sbuf.tile([P, NCELL], F32, tag="pre_t1")
    for p_ in range(4):
        hs = h[:, p_ * NCELL:(p_ + 1) * NCELL]
        nc.vector.tensor_scalar_mul(t0[:], cpt[:, 0:NCELL], float(B_MAT[p_, 0]))
        nc.vector.scalar_tensor_tensor(
            t1[:], cpt[:, 1:1 + NCELL], float(B_MAT[p_, 1]), t0[:], ALU.mult, ALU.add)
        nc.vector.scalar_tensor_tensor(
            t0[:], cpt[:, 2:2 + NCELL], float(B_MAT[p_, 2]), t1[:], ALU.mult, ALU.add)
        nc.vector.scalar_tensor_tensor(
            hs, cpt[:, 3:3 + NCELL], float(B_MAT[p_, 3]), t0[:], ALU.mult, ALU.add)

    # Transpose H (2 chunks of 122 partitions) then pass 2 (contract j).
    for chunk in range(2):
        pt = psum.tile([P, P], F32, tag="pre_psum")
        nc.tensor.transpose(
            out=pt[:122, :], in_=h[:, chunk * 122:(chunk + 1) * 122],
            identity=ident[:])
        h2 = sbuf.tile([122, G, 2], F32, tag="h2")
        nc.vector.tensor_copy(h2[:].rearrange("p a b -> p (a b)"), pt[:122, :])

        g2 = sbuf.tile([122, NCELL, 8], F32, tag="g2")
        w0 = sbuf.tile([122, NCELL, 2], F32, tag="pre_w0")
        w1 = sbuf.tile([122, NCELL, 2], F32, tag="pre_w1")
        for q_ in range(4):
            gs = g2[:, :, q_ * 2:q_ * 2 + 2]
            nc.vector.tensor_scalar_mul(
                w0[:], h2[:, 0:NCELL, :], float(B_MAT[q_, 0]))
            nc.vector.scalar_tensor_tensor(
                w1[:], h2[:, 1:1 + NCELL, :], float(B_MAT[q_, 1]), w0[:],
                ALU.mult, ALU.add)
            nc.vector.scalar_tensor_tensor(
                w0[:], h2[:, 2:2 + NCELL, :], float(B_MAT[q_, 2]), w1[:],
                ALU.mult, ALU.add)
            nc.vector.scalar_tensor_tensor(
                gs, h2[:, 3:3 + NCELL, :], float(B_MAT[q_, 3]), w0[:],
                ALU.mult, ALU.add)

        # Store: partition (p_local, a-1) -> gall[a-1, :, m0 + (q,c)] where
        # m = p*8 + q*2 + c, p = chunk*2 + p_local.
        for p_local in range(2):
            m0 = (chunk * 2 + p_local) * 8
            nc.sync.dma_start(
                out=gall[:, :, m0:m0 + 8],
                in_=g2[p_local * NCELL:(p_local + 1) * NCELL, :, :])


def build_nc(rows, tile_cols, split=True):
    """rows: points per partition per core. tile_cols: list of chunk sizes."""
    nc = bacc.Bacc()
    cpt_d = nc.dram_tensor("cpt", [P, G], F32, kind="ExternalInput")
    ch1_d = nc.dram_tensor("ch1", [P, rows, 2], F32, kind="ExternalInput")
    ch2_d = nc.dram_tensor("ch2", [P, rows, 2], F32, kind="ExternalInput")
    idx_d = nc.dram_tensor("idx", [P, rows, 2], I32, kind="ExternalInput")
    out = nc.dram_tensor("out", [P, 1], F32, kind="ExternalOutput")

    ntiles = len(tile_cols)
    with tile.TileContext(nc) as tc:
        with tc.tile_pool(name="sbuf", bufs=2) as sbuf, \
             tc.tile_pool(name="psum", bufs=1, space="PSUM") as psum, \
             tc.tile_pool(name="dram", bufs=1, space="DRAM") as dram, \
             tc.tile_pool(name="acc", bufs=1) as accp:

            gall = dram.tile([NCELL, NCELL, 64], F32)
            _emit_precompute(nc, tc, sbuf, psum, gall, cpt_d)

            plist = accp.tile([P, ntiles], F32)
            rows = sum(tile_cols)
            gflat = gall[:].rearrange("a b m -> (a b) m")

            # --- one-time index pipeline: e = 61*a + b - 62 as int16, then
            # wrap into dma_gather's index layout: idxall[q%16, q//16]
            # (replicated across all 8 16-partition groups). Point id
            # q = k*128 + p, so q%16 == p%16 and q//16 = 8*k + p//16.
            idxall = accp.tile([P, rows, 8], I16)
            ECH = 512
            for c0 in range(0, rows, ECH):
                cs = min(ECH, rows - c0)
                ixc = sbuf.tile([P, cs, 2], I32, tag="ixc")
                nc.sync.dma_start(out=ixc[:], in_=idx_d[:, c0:c0 + cs, :])
                ixf = sbuf.tile([P, cs, 2], F32, tag="ixf")
                nc.vector.tensor_copy(ixf[:], ixc[:])
                ea = sbuf.tile([P, cs], F32, tag="ea")
                nc.vector.tensor_scalar_mul(ea[:], ixf[:, :, 0], 61.0)
                ef = sbuf.tile([P, cs], F32, tag="ef")
                nc.vector.scalar_tensor_tensor(
                    ef[:], ixf[:, :, 1], -62.0, ea[:], ALU.add, ALU.add)
                e16 = sbuf.tile([P, cs], I16, tag="e16")
                nc.vector.tensor_copy(e16[:], ef[:])
                for g in range(8):
                    nc.gpsimd.dma_start(
                        out=idxall[0:16, c0:c0 + cs, g],
                        in_=e16[16 * g:16 * (g + 1), :])
            for k in range(1, 8):
                nc.gpsimd.dma_start(
                    out=idxall[16 * k:16 * (k + 1), :, :], in_=idxall[0:16, :, :])

            col0 = 0
            for t, T in enumerate(tile_cols):
                c1 = sbuf.tile([P, T, 2], F32, tag="c1")
                c2 = sbuf.tile([P, T, 2], F32, tag="c2")
                sl = np.s_[:, col0:col0 + T, :]
                nc.sync.dma_start(out=c1[:], in_=ch1_d[sl])
                nc.sync.dma_start(out=c2[:], in_=ch2_d[sl])

                gv = sbuf.tile([P, T, 64], F32, tag="gv")
                # descriptor-ring capacity limits one call to ~1024 indices
                for j0 in range(0, T, GSUB):
                    jn = min(GSUB, T - j0)
                    nc.gpsimd.dma_gather(
                        out_ap=gv[:, j0:j0 + jn, :],
                        in_ap=gflat,
                        idxs_ap=idxall[:, col0 + j0:col0 + j0 + jn, :],
                        num_idxs=P * jn,
                        num_idxs_reg=P * jn,
                        elem_size=64,
                    )

                fi = sbuf.tile([P, T, 2], I32, tag="fi")
                nc.vector.tensor_copy(fi[:], c2[:])
                ff = sbuf.tile([P, T, 2], F32, tag="ff")
                nc.vector.tensor_copy(ff[:], fi[:])
                f0 = sbuf.tile([P, T, 2], F32, tag="f0")
                nc.vector.tensor_tensor(f0[:], c2[:], ff[:], ALU.subtract)
                fm = sbuf.tile([P, T, 2], F32, tag="fm")
                nc.vector.tensor_scalar(fm[:], f0[:], 0.0, None, ALU.is_lt)
                f = sbuf.tile([P, T, 2], F32, tag="f")
                nc.vector.tensor_tensor(f[:], f0[:], fm[:], ALU.add)
                x = f[:, :, 0:1]
                y = f[:, :, 1:2]
                xb = x.to_broadcast([P, T, 8])
                yb = y.to_broadcast([P, T, 2])

                u = sbuf.tile([P, T, 8], F32, tag="u")
                v = sbuf.tile([P, T, 8], F32, tag="v")
                # Horner in x over the 4 p-slices of gv: result u[(q,c)]
                nc.vector.tensor_tensor(u[:], gv[:, :, 24:32], xb, ALU.mult)
                nc.vector.tensor_tensor(v[:], u[:], gv[:, :, 16:24], ALU.add)
                nc.vector.tensor_tensor(u[:], v[:], xb, ALU.mult)
                nc.vector.tensor_tensor(v[:], u[:], gv[:, :, 8:16], ALU.add)
                nc.vector.tensor_tensor(u[:], v[:], xb, ALU.mult)
                nc.vector.tensor_tensor(v[:], u[:], gv[:, :, 0:8], ALU.add)

                o = sbuf.tile([P, T, 2], F32, tag="o")
                w = sbuf.tile([P, T, 2], F32, tag="w")
                # Horner in y over the 4 q-slices of v: out[c]
                nc.vector.tensor_tensor(o[:], v[:, :, 6:8], yb, ALU.mult)
                nc.vector.tensor_tensor(w[:], o[:], v[:, :, 4:6], ALU.add)
                nc.vector.tensor_tensor(o[:], w[:], yb, ALU.mult)
                nc.vector.tensor_tensor(w[:], o[:], v[:, :, 2:4], ALU.add)
                nc.vector.tensor_tensor(o[:], w[:], yb, ALU.mult)
                nc.vector.tensor_tensor(w[:], o[:], v[:, :, 0:2], ALU.add)

                d = sbuf.tile([P, T, 2], F32, tag="d")
                nc.vector.tensor_tensor(d[:], c1[:], w[:], ALU.subtract)
                ds = sbuf.tile([P, T, 2], F32, tag="ds")
                nc.vector.scalar_tensor_tensor(
                    ds[:], d[:], 1.0, d[:], ALU.mult, ALU.mult,
                    accum_out=plist[:, t:t + 1])
                col0 += T

            lsum = accp.tile([P, 1], F32)
            nc.vector.tensor_reduce(
                lsum[:], plist[:], axis=mybir.AxisListType.X, op=ALU.add)
            nc.sync.dma_start(out=out[:], in_=lsum[:])
    nc.compile()
    if split:
        _split_multiwait(nc)
    # The runner calls nc.finalize(); Bacc.finalize would re-run compile()
    # after our wait-splitting, so bind the base finalize instead.
    nc.finalize = types.MethodType(bass.Bass.finalize, nc)
    return nc


_NC_CACHE = {}


def _get_nc(rows, tile_cols):
    key = (rows, tuple(tile_cols))
    if key not in _NC_CACHE:
        _NC_CACHE[key] = build_nc(rows, tile_cols)
    return _NC_CACHE[key]


def _split_tiles(rows, tmax=64):
    out = []
    r = rows
    while r > 0:
        out.append(min(tmax, r))
        r -= min(tmax, r)
    return out


def kernel(ch1, ch2, CP_locs, CP_idx):
    n = ch1.shape[0]
    rows = -(-n // (NCORES * P))  # points per partition per core
    n_core = rows * P
    n_pad = n_core * NCORES

    ch1 = np.ascontiguousarray(ch1, dtype=np.float32)
    ch2 = np.ascontiguousarray(ch2, dtype=np.float32)
    CP_locs = np.ascontiguousarray(CP_locs, dtype=np.float32)
    CP_idx = np.ascontiguousarray(CP_idx, dtype=np.int32)

    # Pad with exact-zero-loss points: cell (1,1) at x=y=0 gives
    # out = CP_locs[1,1,:]; set ch1 to the same value.
    if n_pad != n:
        pad = n_pad - n
        ch1 = np.concatenate(
            [ch1, np.broadcast_to(CP_locs[1, 1, :], (pad, 2))], axis=0)
        ch2 = np.concatenate([ch2, np.zeros((pad, 2), np.float32)], axis=0)
        CP_idx = np.concatenate(
            [CP_idx, np.ones((pad, 2), np.int32)], axis=0)

    cpt = np.ascontiguousarray(CP_locs.transpose(1, 2, 0).reshape(P, G))
    ch1s = ch1.reshape(NCORES, P, rows, 2)
    ch2s = ch2.reshape(NCORES, P, rows, 2)
    idxs = CP_idx.reshape(NCORES, P, rows, 2)

    nc = _get_nc(rows, _split_tiles(rows))
    in_maps = [
        {"cpt": cpt, "ch1": ch1s[i], "ch2": ch2s[i], "idx": idxs[i]}
        for i in range(NCORES)
    ]
    res = run_bass_kernel_spmd(nc, in_maps, core_ids=list(range(NCORES)))
    total = np.float64(0.0)
    for i in range(NCORES):
        total += np.sum(res.results[i]["out"].astype(np.float64))
    return np.float32(total)


# revision 26
# speedup vs baseline: 2788.6095x; 2788.6095x over previous
"""Catmull-Rom spline loss kernel for Trainium2 (8 NeuronCores, data-parallel).

Math: out[n,c] = sum_ij wx_i wy_j CP[a+i-1, b+j-1, c] with wx = [x^3,x^2,x,1]@A.
Rewritten in the monomial basis: out[n,c] = sum_{p,q} x^p y^q G_pq[a,b,c] where
G_pq[a,b,c] = sum_ij B[p,i] B[q,j] CP[a-1+i, b-1+j, c] and B = A rows reversed.
G is precomputed on-device from CP (it is cell-indexed: a,b in [1,61]), stored
as a [61,61,32] DRAM table (minor dim = p*8 + q*2 + c). Each point then needs a
single 128-byte row gather (indirect DMA) + polynomial evaluation (DVE).
loss = sum_n |ch1_n - out_n|^2 accumulated per partition, reduced on host.
"""

import os

os.environ.setdefault("MYCRO_LOCAL_CACHE", "1")

import numpy as np

import bass_rust
import concourse.bass as bass
import concourse.mybir as mybir
import concourse.tile as tile
import concourse.bacc as bacc
import types
from concourse.masks import make_identity
from concourse.bass_utils import run_bass_kernel_spmd

F32 = mybir.dt.float32
I32 = mybir.dt.int32
I16 = mybir.dt.int16
ALU = mybir.AluOpType

NCORES = 8
P = 128
G = 64
NCELL = 61  # valid index range [1, 61] -> 61 cells per axis
GSUB = 8  # gather sub-call rows (1024 indices) - SWDGE descriptor ring limit

# Catmull-Rom basis (same as reference.py); B[p] = A[3-p] so that
# wx_i = sum_p B[p, i] * x^p.
A_MAT = np.array(
    [[-0.5, 1.5, -1.5, 0.5],
     [1.0, -2.5, 2.0, -0.5],
     [-0.5, 0.0, 0.5, 0.0],
     [0.0, 1.0, 0.0, 0.0]], dtype=np.float64)
B_MAT = A_MAT[::-1, :]

_MAX_WAITS = 1


def _split_multiwait(nc, max_waits=_MAX_WAITS):
    """The walrus snapshot here rejects instructions carrying more than one
    sync wait; move extra waits onto injected same-engine NoOps."""
    n_split = 0
    for bb in nc.main_func.blocks:
        insts = bb.instructions
        new = []
        for ins in insts:
            si = ins.sync_info
            waits = list(si.on_wait) if si and si.on_wait else []
            if len(waits) > max_waits:
                extra, keep = waits[:-max_waits], waits[-max_waits:]
                for k in range(0, len(extra), max_waits):
                    nop = mybir.InstDrain(
                        name=f"{ins.name}-wsplit{k}", ins=[], outs=[])
                    nop.engine = ins.engine
                    nop.sync_info = bass_rust.SyncInfo(
                        on_wait=extra[k:k + max_waits], on_update=[])
                    new.append(nop)
                ins.sync_info = bass_rust.SyncInfo(
                    on_wait=keep,
                    on_update=list(si.on_update) if si.on_update else [])
                n_split += 1
            new.append(ins)
        insts[:] = new
    return n_split


def _emit_precompute(nc, tc, sbuf, psum, gall, cpt_d):  # noqa: C901
    """Build G[61,61,32] fp32 in DRAM from the cpt input ([128,64] = CP
    transposed so partition p = 2*s + c holds row-series CP[:, s, c])."""
    ident = sbuf.tile([P, P], F32, tag="ident")
    make_identity(nc, ident[:])

    # zero-fill the 256B-row padding (cols 32:64) once; the gather reads
    # full rows and the simulator requires finite values.
    zt = sbuf.tile([NCELL, NCELL * 16], F32, tag="zpad")
    nc.gpsimd.memset(zt[:], 0.0)
    nc.sync.dma_start(
        out=gall[:, :, 32:48],
        in_=zt[:].rearrange("p (b m) -> p b m", m=16))
    nc.sync.dma_start(
        out=gall[:, :, 48:64],
        in_=zt[:].rearrange("p (b m) -> p b m", m=16))

    cpt = sbuf.tile([P, G], F32, tag="cpt")
    nc.sync.dma_start(out=cpt[:], in_=cpt_d[:])

    # Pass 1 (contract row-offset i): H[(s,c), p*61 + (a-1)]
    h = sbuf.tile([P, 4 * NCELL], F32, tag="h")
    t0 = sbuf.tile([P, NCELL], F32, tag="pre_t0")
    t1 = sbuf.tile([P, NCELL], F32, tag="pre_t1")
    for p_ in range(4):
        hs = h[:, p_ * NCELL:(p_ + 1) * NCELL]
        nc.vector.tensor_scalar_mul(t0[:], cpt[:, 0:NCELL], float(B_MAT[p_, 0]))
        nc.vector.scalar_tensor_tensor(
            t1[:], cpt[:, 1:1 + NCELL], float(B_MAT[p_, 1]), t0[:], ALU.mult, ALU.add)
        nc.vector.scalar_tensor_tensor(
            t0[:], cpt[:, 2:2 + NCELL], float(B_MAT[p_, 2]), t1[:], ALU.mult, ALU.add)
        nc.vector.scalar_tensor_tensor(
            hs, cpt[:, 3:3 + NCELL], float(B_MAT[p_, 3]), t0[:], ALU.mult, ALU.add)

    # Transpose H (2 chunks of 122 partitions) then pass 2 (contract j).
    for chunk in range(2):
        pt = psum.tile([P, P], F32, tag="pre_psum")
        nc.tensor.transpose(
            out=pt[:122, :], in_=h[:, chunk * 122:(chunk + 1) * 122],
            identity=ident[:])
        h2 = sbuf.tile([122, G, 2], F32, tag="h2")
        nc.vector.tensor_copy(h2[:].rearrange("p a b -> p (a b)"), pt[:122, :])

        g2 = sbuf.tile([122, NCELL, 8], F32, tag="g2")
        w0 = sbuf.tile([122, NCELL, 2], F32, tag="pre_w0")
        w1 = sbuf.tile([122, NCELL, 2], F32, tag="pre_w1")
        for q_ in range(4):
            gs = g2[:, :, q_ * 2:q_ * 2 + 2]
            nc.vector.tensor_scalar_mul(
                w0[:], h2[:, 0:NCELL, :], float(B_MAT[q_, 0]))
            nc.vector.scalar_tensor_tensor(
                w1[:], h2[:, 1:1 + NCELL, :], float(B_MAT[q_, 1]), w0[:],
                ALU.mult, ALU.add)
            nc.vector.scalar_tensor_tensor(
                w0[:], h2[:, 2:2 + NCELL, :], float(B_MAT[q_, 2]), w1[:],
                ALU.mult, ALU.add)
            nc.vector.scalar_tensor_tensor(
                gs, h2[:, 3:3 + NCELL, :], float(B_MAT[q_, 3]), w0[:],
                ALU.mult, ALU.add)

        # Store: partition (p_local, a-1) -> gall[a-1, :, m0 + (q,c)] where
        # m = p*8 + q*2 + c, p = chunk*2 + p_local.
        for p_local in range(2):
            m0 = (chunk * 2 + p_local) * 8
            nc.sync.dma_start(
                out=gall[:, :, m0:m0 + 8],
                in_=g2[p_local * NCELL:(p_local + 1) * NCELL, :, :])




def _dma_gather_raw(gp, out_ap, in_ap, idxs_ap, num_idxs, elem_size, elem_step):
    """dma_gather minus the elem_size_bytes %% 256 restriction (which only
    the transpose/xbar path needs; the non-transpose ucode supports any
    length as long as the source STRIDE is a multiple of 256B)."""
    assert in_ap.ap[0][0] == elem_step
    stride_bytes = elem_step * mybir.dt.size(in_ap.dtype)
    stride_bytes_256 = stride_bytes // 256
    assert stride_bytes_256 * 256 == stride_bytes and stride_bytes_256 < 256
    _in_ap = gp.lower_ap_dma(in_ap, for_custom_bir_dma=True)
    _idxs_ap = gp.lower_ap(idxs_ap)
    _out_ap = gp.lower_ap(out_ap)
    return gp.add_instruction(
        mybir.InstDMAGatherAnt(
            name=gp.bass.get_next_instruction_name(),
            ins=[*_in_ap, _idxs_ap, gp.lower_val_access(gp.to_reg(num_idxs))],
            outs=[_out_ap],
            transpose=False,
            num_idxs=num_idxs,
            elem_size=elem_size,
            stride_bytes_256=stride_bytes_256,
            gen_mode=0,
            single_packet=True,
            queue_num=0,
            sbuf_tokens_per_rank=0,
            sbuf_free_dim_per_rank=0,
            sbuf_free_dim_pad_per_rank=0,
            sbuf_byte_offset=0,
        )
    )

def build_nc(rows, tile_cols, split=True):
    skip_gather = os.environ.get("K_SKIP_GATHER") == "1"
    skip_compute = os.environ.get("K_SKIP_COMPUTE") == "1"
    """rows: points per partition per core. tile_cols: list of chunk sizes."""
    nc = bacc.Bacc()
    cpt_d = nc.dram_tensor("cpt", [P, G], F32, kind="ExternalInput")
    ch1_d = nc.dram_tensor("ch1", [P, rows, 2], F32, kind="ExternalInput")
    ch2_d = nc.dram_tensor("ch2", [P, rows, 2], F32, kind="ExternalInput")
    idx_d = nc.dram_tensor("idx", [P, rows, 2], I32, kind="ExternalInput")
    out = nc.dram_tensor("out", [P, 1], F32, kind="ExternalOutput")

    ntiles = len(tile_cols)
    with tile.TileContext(nc) as tc:
        with tc.tile_pool(name="sbuf", bufs=2) as sbuf, \
             tc.tile_pool(name="psum", bufs=1, space="PSUM") as psum, \
             tc.tile_pool(name="dram", bufs=1, space="DRAM") as dram, \
             tc.tile_pool(name="acc", bufs=1) as accp:

            gall = dram.tile([NCELL, NCELL, 64], F32)
            _emit_precompute(nc, tc, sbuf, psum, gall, cpt_d)

            plist = accp.tile([P, ntiles], F32)
            rows = sum(tile_cols)
            gflat = gall[:, :, 0:32].rearrange("a b m -> (a b) m")

            # --- one-time index pipeline: e = 61*a + b - 62 as int16, then
            # wrap into dma_gather's index layout: idxall[q%16, q//16]
            # (replicated across all 8 16-partition groups). Point id
            # q = k*128 + p, so q%16 == p%16 and q//16 = 8*k + p//16.
            idxall = accp.tile([P, rows, 8], I16)
            ECH = 512
            for c0 in range(0, rows, ECH):
                cs = min(ECH, rows - c0)
                ixc = sbuf.tile([P, cs, 2], I32, tag="ixc")
                nc.sync.dma_start(out=ixc[:], in_=idx_d[:, c0:c0 + cs, :])
                ixf = sbuf.tile([P, cs, 2], F32, tag="ixf")
                nc.vector.tensor_copy(ixf[:], ixc[:])
                ea = sbuf.tile([P, cs], F32, tag="ea")
                nc.vector.tensor_scalar_mul(ea[:], ixf[:, :, 0], 61.0)
                ef = sbuf.tile([P, cs], F32, tag="ef")
                nc.vector.scalar_tensor_tensor(
                    ef[:], ixf[:, :, 1], -62.0, ea[:], ALU.add, ALU.add)
                e16 = sbuf.tile([P, cs], I16, tag="e16")
                nc.vector.tensor_copy(e16[:], ef[:])
                for g in range(8):
                    nc.sync.dma_start(
                        out=idxall[0:16, c0:c0 + cs, g],
                        in_=e16[16 * g:16 * (g + 1), :])
            for k in range(1, 8):
                nc.sync.dma_start(
                    out=idxall[16 * k:16 * (k + 1), :, :], in_=idxall[0:16, :, :])

            col0 = 0
            for t, T in enumerate(tile_cols):
                c1 = sbuf.tile([P, T, 2], F32, tag="c1")
                c2 = sbuf.tile([P, T, 2], F32, tag="c2")
                sl = np.s_[:, col0:col0 + T, :]
                nc.sync.dma_start(out=c1[:], in_=ch1_d[sl])
                nc.sync.dma_start(out=c2[:], in_=ch2_d[sl])

                gv = sbuf.tile([P, T, 32], F32, tag="gv")
                # descriptor-ring capacity limits one call to ~1024 indices
                if skip_gather:
                    nc.vector.memset(gv[:], 1.0)
                for j0 in ([] if skip_gather else range(0, T, GSUB)):
                    jn = min(GSUB, T - j0)
                    _dma_gather_raw(
                        nc.gpsimd,
                        out_ap=gv[:, j0:j0 + jn, :],
                        in_ap=gflat,
                        idxs_ap=idxall[:, col0 + j0:col0 + j0 + jn, :],
                        num_idxs=P * jn,
                        elem_size=32,
                        elem_step=64,
                    )

                if skip_compute:
                    nc.vector.tensor_reduce(
                        plist[:, t:t + 1], gv[:, :, 0],
                        axis=mybir.AxisListType.X, op=ALU.add)
                    col0 += T
                    continue
                fi = sbuf.tile([P, T, 2], I32, tag="fi")
                nc.scalar.activation(fi[:], c2[:], mybir.ActivationFunctionType.Copy)
                ff = sbuf.tile([P, T, 2], F32, tag="ff")
                nc.scalar.activation(ff[:], fi[:], mybir.ActivationFunctionType.Copy)
                f0 = sbuf.tile([P, T, 2], F32, tag="f0")
                nc.vector.tensor_tensor(f0[:], c2[:], ff[:], ALU.subtract)
                f = sbuf.tile([P, T, 2], F32, tag="f")
                nc.vector.scalar_tensor_tensor(
                    f[:], f0[:], 0.0, f0[:], ALU.is_lt, ALU.add)
                x = f[:, :, 0:1]
                y = f[:, :, 1:2]
                xb = x.to_broadcast([P, T, 8])
                yb = y.to_broadcast([P, T, 2])

                u = sbuf.tile([P, T, 8], F32, tag="u")
                v = sbuf.tile([P, T, 8], F32, tag="v")
                # Horner in x over the 4 p-slices of gv: result u[(q,c)]
                nc.vector.tensor_tensor(u[:], gv[:, :, 24:32], xb, ALU.mult)
                nc.vector.tensor_tensor(v[:], u[:], gv[:, :, 16:24], ALU.add)
                nc.vector.tensor_tensor(u[:], v[:], xb, ALU.mult)
                nc.vector.tensor_tensor(v[:], u[:], gv[:, :, 8:16], ALU.add)
                nc.vector.tensor_tensor(u[:], v[:], xb, ALU.mult)
                nc.vector.tensor_tensor(v[:], u[:], gv[:, :, 0:8], ALU.add)

                o = sbuf.tile([P, T, 2], F32, tag="o")
                w = sbuf.tile([P, T, 2], F32, tag="w")
                # Horner in y over the 4 q-slices of v: out[c]
                nc.vector.tensor_tensor(o[:], v[:, :, 6:8], yb, ALU.mult)
                nc.vector.tensor_tensor(w[:], o[:], v[:, :, 4:6], ALU.add)
                nc.vector.tensor_tensor(o[:], w[:], yb, ALU.mult)
                nc.vector.tensor_tensor(w[:], o[:], v[:, :, 2:4], ALU.add)
                nc.vector.tensor_tensor(o[:], w[:], yb, ALU.mult)
                nc.vector.tensor_tensor(w[:], o[:], v[:, :, 0:2], ALU.add)

                d = sbuf.tile([P, T, 2], F32, tag="d")
                nc.vector.tensor_tensor(d[:], c1[:], w[:], ALU.subtract)
                ds = sbuf.tile([P, T, 2], F32, tag="ds")
                nc.vector.scalar_tensor_tensor(
                    ds[:], d[:], 1.0, d[:], ALU.mult, ALU.mult,
                    accum_out=plist[:, t:t + 1])
                col0 += T

            lsum = accp.tile([P, 1], F32)
            nc.vector.tensor_reduce(
                lsum[:], plist[:], axis=mybir.AxisListType.X, op=ALU.add)
            nc.sync.dma_start(out=out[:], in_=lsum[:])
    nc.compile()
    if split:
        _split_multiwait(nc)
    # The runner calls nc.finalize(); Bacc.finalize would re-run compile()
    # after our wait-splitting, so bind the base finalize instead.
    nc.finalize = types.MethodType(bass.Bass.finalize, nc)
    return nc


_NC_CACHE = {}


def _get_nc(rows, tile_cols):
    key = (rows, tuple(tile_cols))
    if key not in _NC_CACHE:
        _NC_CACHE[key] = build_nc(rows, tile_cols)
    return _NC_CACHE[key]


def _split_tiles(rows, tmax=128):
    out = []
    r = rows
    while r > 0:
        out.append(min(tmax, r))
        r -= min(tmax, r)
    return out


def kernel(ch1, ch2, CP_locs, CP_idx):
    n = ch1.shape[0]
    rows = -(-n // (NCORES * P))  # points per partition per core
    n_core = rows * P
    n_pad = n_core * NCORES

    ch1 = np.ascontiguousarray(ch1, dtype=np.float32)
    ch2 = np.ascontiguousarray(ch2, dtype=np.float32)
    CP_locs = np.ascontiguousarray(CP_locs, dtype=np.float32)
    CP_idx = np.ascontiguousarray(CP_idx, dtype=np.int32)

    # Pad with exact-zero-loss points: cell (1,1) at x=y=0 gives
    # out = CP_locs[1,1,:]; set ch1 to the same value.
    if n_pad != n:
        pad = n_pad - n
        ch1 = np.concatenate(
            [ch1, np.broadcast_to(CP_locs[1, 1, :], (pad, 2))], axis=0)
        ch2 = np.concatenate([ch2, np.zeros((pad, 2), np.float32)], axis=0)
        CP_idx = np.concatenate(
            [CP_idx, np.ones((pad, 2), np.int32)], axis=0)

    cpt = np.ascontiguousarray(CP_locs.transpose(1, 2, 0).reshape(P, G))
    ch1s = ch1.reshape(NCORES, P, rows, 2)
    ch2s = ch2.reshape(NCORES, P, rows, 2)
    idxs = CP_idx.reshape(NCORES, P, rows, 2)

    nc = _get_nc(rows, _split_tiles(rows))
    in_maps = [
        {"cpt": cpt, "ch1": ch1s[i], "ch2": ch2s[i], "idx": idxs[i]}
        for i in range(NCORES)
    ]
    res = run_bass_kernel_spmd(nc, in_maps, core_ids=list(range(NCORES)))
    total = np.float64(0.0)
    for i in range(NCORES):
        total += np.sum(res.results[i]["out"].astype(np.float64))
    return np.float32(total)
